# revision 23
# baseline (speedup 1.0000x reference)
"""DigitCaps dynamic-routing kernel for Trainium2 (8 NeuronCores, SPMD).

Problem:  u = einsum('bri,rcio->brco', x, W[0]);  3 routing iterations
          (softmax over capsules, weighted sum over routes, squash,
          agreement update);  returns v [B, C, OC].

Shapes: B=256, R=1152, C=10, IC=8, OC=16.  Batch-sharded 8 ways (BL=32
per core, zero cross-core communication).

Design notes (per core):
 - u-phase: r in 72 chunks of G=16 routes; block-diag x (xbd) gives
   128-partition matmuls so each chunk's u lands as 4x [128,160] PSUM
   tiles; u resident in SBUF as bf16 [p=(b8,g16), f=(bg4,k72,c10,o16)]
   (bg-major so the delta-pass reads are contiguous).
 - iter-1 shortcut: c uniform = 1/10, so s1 = 0.1*sum_r u comes from 72
   accumulating matmuls with compact x (xp) as lhsT.  wt/xp load in one
   big DMA each up-front, so v1 is ready ~20us in and iteration 2
   overlaps the u-phase.
 - delta (b-update, sum_o u*v): DVE fp16 multiply at 2x mode + fp16
   add-tree (16->8->4->2->1), also at 2x.  The old tensor_reduce ran at
   1x (InstTensorReduce supports no DVE perf modes); the tree is ~1.7x
   faster end to end.  Logits accumulate in fp16 (|b| <~ 60, quantum
   ~0.03 -- checked against the 2e-2 gate).
 - u PSUM->SBUF copies run on scalar+gpsimd so the DVE is free for the
   overlapped iteration-2 work.
 - squash: sqrt as exp(0.5*ln(n2+eps^2)); ln/exp/copy share one
   activation table (natural_log_exp_and_others) so the scalar engine
   never reloads tables mid-kernel (the old sqrt<->exp alternation cost
   ~1.3us per switch).
 - s-pass: block-diag c (cbd = c x batch-mask, with 1/softmax-sum folded
   into the mask) as lhsT against resident u, PSUM-accumulated over all
   72 chunks; (c,c') diagonal extracted with small DMAs.  cbd builds go
   to gpsimd except the final tail unit (vector, to shorten the tail).
 - PSUM budget (8 banks): 4x u-tiles + s1 + 2 paired s-tiles + vb.
"""

import sys

sys.path.insert(0, "/opt/trn_rl_repo")

from contextlib import ExitStack

import ml_dtypes
import numpy as np

import concourse.bass as bass
import concourse.tile as tile
from concourse import bacc, mybir
from concourse.bass_utils import run_bass_kernel_spmd

BF16 = mybir.dt.bfloat16
F16 = mybir.dt.float16
F32 = mybir.dt.float32
AF = mybir.ActivationFunctionType
ALU = mybir.AluOpType
AX = mybir.AxisListType

B, R, C, IC, OC = 256, 1152, 10, 8, 16
NCORES = 8
BL = B // NCORES  # 32 batches per core
G = 16  # routes per chunk
NBG = BL // 8  # 4 b-groups of 8
CO = C * OC  # 160
EPS2 = 1e-16
NPBF = ml_dtypes.bfloat16

# Set by tests to shrink the problem for simulation; full size by default.
_R_OVERRIDE = None
_DEBUG = False


def _nchunks(r=None):
    r = r if r is not None else (_R_OVERRIDE or R)
    assert r % G == 0
    return r // G


class _Kern:
    def __init__(self, ctx, tc, K):
        self.nc = tc.nc
        self.K = K
        self.KH = K // 2
        self.per = ctx.enter_context(tc.tile_pool(name="per", bufs=1))
        self.xbdp = ctx.enter_context(tc.tile_pool(name="xbdp", bufs=2))
        self.upsum = ctx.enter_context(
            tc.tile_pool(name="upsum", bufs=4, space="PSUM")
        )
        self.s1psum = ctx.enter_context(
            tc.tile_pool(name="s1psum", bufs=1, space="PSUM")
        )
        self.spsum = ctx.enter_context(
            tc.tile_pool(name="spsum", bufs=1, space="PSUM")
        )
        self.vbpsum = ctx.enter_context(
            tc.tile_pool(name="vbpsum", bufs=1, space="PSUM")
        )
        self.tmpp = ctx.enter_context(tc.tile_pool(name="tmpp", bufs=1))
        self.trp = ctx.enter_context(tc.tile_pool(name="trp", bufs=1))
        self.cbdp = ctx.enter_context(tc.tile_pool(name="cbdp", bufs=3))
        self.cxp = ctx.enter_context(tc.tile_pool(name="cxp", bufs=3))
        self.small = ctx.enter_context(tc.tile_pool(name="small", bufs=2))

        K_, KH = K, self.KH
        per = self.per
        self.u1 = per.tile([128, NBG * K_ * CO], BF16, name="u1")
        self.u1v = self.u1[:].rearrange("p (b k x) -> p b k x", b=NBG, k=K_)
        self.logits = per.tile([128, NBG * K_ * C], F16, name="logits")
        self.logv = self.logits[:].rearrange(
            "p (b k c) -> p b k c", b=NBG, k=K_
        )
        self.wt_t = per.tile([128, K_ * CO], BF16, name="wt_t")
        self.xp_t = per.tile([128, K_ * BL], BF16, name="xp_t")
        self.sel_t = per.tile([8, 128], BF16, name="sel_t")
        self.msk_t = per.tile([128, 8], BF16, name="msk_t")
        self.s_sb = per.tile([BL, CO], F32, name="s_sb")
        self.s8 = per.tile([8, NBG * CO], F32, name="s8")
        self.vbf8 = per.tile([8, NBG * CO], BF16, name="vbf8")
        self.vb_a = per.tile([128, NBG * CO], BF16, tag="vb_a", name="vb_a")
        self.vb_b = per.tile([128, NBG * CO], BF16, tag="vb_b", name="vb_b")
        self.s84a = per.tile([8, NBG * CO], F32, name="s84a")
        self.s84b = per.tile([8, NBG * CO], F32, name="s84b")
        self.v84 = per.tile([8, NBG * CO], F32, name="v84")
        self.vbf8b = per.tile([8, NBG * CO], BF16, name="vbf8b")
        self.epsb = per.tile([128, 1], F32, name="epsb")
        self.nc.vector.memset(self.epsb[:], EPS2)
        self.sps = {}  # (it, pair) -> [80, 2*CO] PSUM tile
        self.cbds = {}  # (it, bg, kh) -> cbd view
        self.sbg = {}  # (it, bg) -> extracted s tile

    # -- squash ----------------------------------------------------------
    def squash(self, s_ap, v_ap, pre, np_, nseg):
        """v = squash(pre*s) for [np_, nseg*OC] tiles at partition base 0."""
        nc, pool = self.nc, self.small
        sq = pool.tile([np_, nseg * OC], F32, tag="sq", bufs=1)
        if pre == 1.0:
            nc.vector.tensor_tensor(out=sq[:], in0=s_ap, in1=s_ap, op=ALU.mult)
        else:
            nc.vector.scalar_tensor_tensor(
                out=sq[:], in0=s_ap, scalar=pre * pre, in1=s_ap,
                op0=ALU.mult, op1=ALU.mult,
            )
        n2 = pool.tile([np_, nseg], F32, tag="n2", bufs=1)
        nc.vector.reduce_sum(
            out=n2[:], in_=sq[:].rearrange("p (c o) -> p c o", c=nseg), axis=AX.X
        )
        # nrm = sqrt(n2 + EPS2) via exp(0.5*ln(.)): stays in one act table
        nrm = pool.tile([np_, nseg], F32, tag="nrm", bufs=1)
        nc.scalar.activation(nrm[:], n2[:], AF.Ln, bias=self.epsb[:np_])
        nc.scalar.activation(nrm[:], nrm[:], AF.Exp, scale=0.5)
        t1 = pool.tile([np_, nseg], F32, tag="t1", bufs=1)
        nc.vector.tensor_scalar(
            out=t1[:], in0=n2[:], scalar1=1.0, scalar2=None, op0=ALU.add
        )
        den = pool.tile([np_, nseg], F32, tag="den", bufs=1)
        nc.vector.tensor_tensor(out=den[:], in0=nrm[:], in1=t1[:], op=ALU.mult)
        rden = pool.tile([np_, nseg], F32, tag="rden", bufs=1)
        nc.vector.reciprocal_approx_fast(rden[:], den[:])
        sc = pool.tile([np_, nseg], F32, tag="sc", bufs=1)
        if pre == 1.0:
            nc.vector.tensor_tensor(
                out=sc[:], in0=n2[:], in1=rden[:], op=ALU.mult
            )
        else:
            nc.vector.scalar_tensor_tensor(
                out=sc[:], in0=n2[:], scalar=pre, in1=rden[:],
                op0=ALU.mult, op1=ALU.mult,
            )
        nc.vector.tensor_tensor(
            out=v_ap.rearrange("p (c o) -> p c o", c=nseg),
            in0=s_ap.rearrange("p (c o) -> p c o", c=nseg),
            in1=sc[:].unsqueeze(2).broadcast_to([np_, nseg, OC]),
            op=ALU.mult,
        )

    # -- one routing unit: delta ----------------------------------------
    def unit_delta(self, it, bg, kh, vb):
        """fp16 delta = sum_o u*v via 2x multiply + 2x add-tree."""
        nc, KH = self.nc, self.KH
        ks = kh * KH
        tmpt = self.tmpp.tile([128, KH * CO], F16, tag="tmp")
        nc.vector.tensor_tensor(
            out=tmpt[:].rearrange("p (k x) -> p k x", k=KH),
            in0=self.u1v[:, bg, ks : ks + KH],
            in1=vb[:, bg * CO : (bg + 1) * CO]
            .unsqueeze(1)
            .broadcast_to([128, KH, CO]),
            op=ALU.mult,
        )
        tv = tmpt[:].rearrange("p (s o) -> p s o", o=16)
        t8 = self.trp.tile([128, KH * C * 8], F16, tag="t8")
        t8v = t8[:].rearrange("p (s o) -> p s o", o=8)
        nc.vector.tensor_tensor(
            out=t8v, in0=tv[:, :, 0:8], in1=tv[:, :, 8:16], op=ALU.add
        )
        t4 = self.trp.tile([128, KH * C * 4], F16, tag="t4")
        t4v = t4[:].rearrange("p (s o) -> p s o", o=4)
        nc.vector.tensor_tensor(
            out=t4v, in0=t8v[:, :, 0:4], in1=t8v[:, :, 4:8], op=ALU.add
        )
        t2 = self.trp.tile([128, KH * C * 2], F16, tag="t2")
        t2v = t2[:].rearrange("p (s o) -> p s o", o=2)
        nc.vector.tensor_tensor(
            out=t2v, in0=t4v[:, :, 0:2], in1=t4v[:, :, 2:4], op=ALU.add
        )
        lh = self.logv[:, bg, ks : ks + KH].rearrange("p k c -> p (k c)")
        if it == 2:
            nc.vector.tensor_tensor(
                out=lh, in0=t2v[:, :, 0], in1=t2v[:, :, 1], op=ALU.add
            )
        else:
            dtm = self.trp.tile([128, KH * C], F16, tag="dtm")
            nc.vector.tensor_tensor(
                out=dtm[:], in0=t2v[:, :, 0], in1=t2v[:, :, 1], op=ALU.add
            )
            nc.vector.tensor_tensor(out=lh, in0=lh, in1=dtm[:], op=ALU.add)

    # -- one routing unit: softmax + block-diag c ------------------------
    def unit_soft(self, it, bg, kh, tail=False):
        nc, KH = self.nc, self.KH
        ks = kh * KH
        lh = self.logv[:, bg, ks : ks + KH]
        ch = self.cxp.tile([128, KH * C], BF16, tag="ch")
        chv = ch[:].rearrange("p (k c) -> p k c", k=KH)
        nc.scalar.activation(chv, lh, AF.Exp)
        sume = self.small.tile([128, KH], F32, tag="sume")
        nc.vector.reduce_sum(out=sume[:], in_=chv, axis=AX.X)
        rs = self.small.tile([128, KH], F32, tag="rs")
        nc.vector.reciprocal_approx_fast(rs[:], sume[:])
        rsb = self.small.tile([128, KH], BF16, tag="rsb")
        nc.scalar.copy(rsb[:], rs[:])
        # fold 1/sum into the batch mask, then build block-diag c
        eng = nc.vector if tail else nc.gpsimd
        msk2 = self.small.tile([128, KH * 8], BF16, tag="msk2")
        m2v = msk2[:].rearrange("p (k e) -> p k e", k=KH)
        eng.tensor_tensor(
            out=m2v,
            in0=self.msk_t[:].unsqueeze(1).broadcast_to([128, KH, 8]),
            in1=rsb[:].unsqueeze(2).broadcast_to([128, KH, 8]),
            op=ALU.mult,
        )
        cbd_t = self.cbdp.tile([128, KH * C * 8], BF16, tag="cbd")
        cbdv = cbd_t[:].rearrange("p (k c e) -> p k c e", k=KH, c=C)
        eng.tensor_tensor(
            out=cbdv,
            in0=chv.unsqueeze(3).broadcast_to([128, KH, C, 8]),
            in1=m2v.unsqueeze(2).broadcast_to([128, KH, C, 8]),
            op=ALU.mult,
        )
        self.cbds[(it, bg, kh)] = cbdv

    # -- one routing unit: s-pass matmuls --------------------------------
    # PSUM rule (found the hard way): only ONE open accumulation group per
    # bank -- a start=True while another group in the same bank is still
    # open wipes that group's partials.  Each (bg, kh) chain is therefore
    # its own closed start..stop group; the kh0 result is copied to SBUF
    # before the kh1 groups reuse the bank, and the halves are added there.
    def unit_smm(self, it, bg, kh):
        nc, KH = self.nc, self.KH
        ks = kh * KH
        pair = bg // 2
        if (it, pair) not in self.sps:
            self.sps[(it, pair)] = self.spsum.tile(
                [80, 2 * CO], F32, tag=f"sps{pair}", name=f"sps{it}{pair}"
            )
        half = bg % 2
        sps = self.sps[(it, pair)][:, half * CO : (half + 1) * CO]
        cbdv = self.cbds[(it, bg, kh)]
        for kk in range(KH):
            nc.tensor.matmul(
                sps,
                lhsT=cbdv[:, kk].rearrange("p c e -> p (c e)"),
                rhs=self.u1v[:, bg, ks + kk],
                start=(kk == 0),
                stop=(kk == KH - 1),
            )

    def half_copy(self, it, pr):
        """copy a bg-pair's kh0 s-partials out of PSUM so the bank can host
        the kh1 accumulation groups."""
        h0 = self.small.tile([80, 2 * CO], F32, tag=f"half{pr}", bufs=1)
        self.nc.scalar.copy(h0[:], self.sps[(it, pr)][:])
        self.halfs = getattr(self, "halfs", {})
        self.halfs[(it, pr)] = h0

    def pair_tail(self, it, pr):
        """kh1 + kh0 combine (DVE add straight from PSUM) and (c,c')
        diagonal extraction for a bg-pair."""
        nc = self.nc
        sfull = self.small.tile([80, 2 * CO], F32, tag="sfull")
        nc.vector.tensor_tensor(
            out=sfull[:],
            in0=self.sps[(it, pr)][:],
            in1=self.halfs[(it, pr)][:],
            op=ALU.add,
        )
        s84 = self.s84a if it == 2 else self.s84b
        s84v = s84[:].rearrange("p (b c o) -> p b c o", b=NBG, c=C)
        sfv = sfull[:].rearrange("p (h c o) -> p h c o", h=2, c=C)
        qs = (nc.sync, nc.gpsimd, nc.scalar)
        for c in range(C):
            qs[c % 3].dma_start(
                out=s84v[:, 2 * pr : 2 * pr + 2, c],
                in_=sfv[c * 8 : (c + 1) * 8, :, c],
            )

    def pair_v(self, pr, vb_dst):
        """squash one bg-pair's extracted s -> broadcast into vb (two bgs
        batched: two Ln/Exp pairs per iteration instead of four)."""
        nc = self.nc
        sl = slice(2 * pr * CO, (2 * pr + 2) * CO)
        self.squash(
            self.s84a[:, sl], self.vbf8b[:, sl], pre=1.0, np_=8, nseg=2 * C
        )
        for bg in (2 * pr, 2 * pr + 1):
            vbp = self.vbpsum.tile([128, CO], F32, tag="vbp")
            nc.tensor.matmul(
                vbp[:],
                lhsT=self.sel_t[:],
                rhs=self.vbf8b[:, bg * CO : (bg + 1) * CO],
                start=True,
                stop=True,
            )
            nc.scalar.copy(vb_dst[:, bg * CO : (bg + 1) * CO], vbp[:])

    def final_v(self, pr, out_d):
        """squash one bg-pair of iteration 3 straight to the DRAM output."""
        nc = self.nc
        sl = slice(2 * pr * CO, (2 * pr + 2) * CO)
        self.squash(
            self.s84b[:, sl], self.v84[:, sl], pre=1.0, np_=8, nseg=2 * C
        )
        for bg in (2 * pr, 2 * pr + 1):
            nc.sync.dma_start(
                out=out_d[bg * 8 : (bg + 1) * 8, :],
                in_=self.v84[:, bg * CO : (bg + 1) * CO],
            )


def _body(ctx, tc, xbd_d, wt_d, xp_d, sel_d, msk_d, out_d, K, dbg=None):
    nc = tc.nc
    kn = _Kern(ctx, tc, K)
    KH = kn.KH

    # wt (halves, for an early s1 start) on the scalar queue; xbd streams
    # on sync; all small early DMAs on gpsimd so nothing queues behind the
    # big streams (per-queue FIFO).
    half = K // 2 * CO
    # warm both activation tables (ln-set / exp-set) before the hot path
    nc.scalar.activation(kn.epsb[:8], kn.epsb[:8], AF.Ln)
    nc.scalar.activation(kn.epsb[:8], kn.epsb[:8], AF.Exp)
    nc.vector.memset(kn.epsb[:], EPS2)
    nc.scalar.dma_start(out=kn.wt_t[:, :half], in_=wt_d[:, :half])
    nc.scalar.dma_start(out=kn.wt_t[:, half:], in_=wt_d[:, half:])
    nc.gpsimd.dma_start(out=kn.xp_t[:], in_=xp_d)
    nc.gpsimd.dma_start(out=kn.sel_t[:], in_=sel_d)
    nc.gpsimd.dma_start(out=kn.msk_t[:], in_=msk_d)

    # ---------------- iteration-1 s (emitted mid-u-phase) ---------------
    def emit_s1():
        s1ps = kn.s1psum.tile([BL, CO], F32, tag="s1")
        for k in range(K):
            nc.tensor.matmul(
                s1ps[:],
                lhsT=kn.xp_t[:, k * BL : (k + 1) * BL],
                rhs=kn.wt_t[:, k * CO : (k + 1) * CO],
                start=(k == 0),
                stop=(k == K - 1),
            )
        nc.scalar.copy(kn.s_sb[:], s1ps[:])
        for bg in range(NBG):
            nc.gpsimd.dma_start(
                out=kn.s8[:, bg * CO : (bg + 1) * CO],
                in_=kn.s_sb[bg * 8 : (bg + 1) * 8, :],
            )
        kn.squash(kn.s8[:], kn.vbf8[:], pre=1.0 / C, np_=8, nseg=NBG * C)
        for bg in range(NBG):
            vbp = kn.vbpsum.tile([128, CO], F32, tag="vbp")
            nc.tensor.matmul(
                vbp[:],
                lhsT=kn.sel_t[:],
                rhs=kn.vbf8[:, bg * CO : (bg + 1) * CO],
                start=True,
                stop=True,
            )
            nc.scalar.copy(kn.vb_a[:, bg * CO : (bg + 1) * CO], vbp[:])

    # ---------------- u-phase (iteration-2 kh0 work interleaved) --------
    assert K % 4 == 0 or K < 4
    KB = 4 if K % 4 == 0 else 1
    # emission points (in chunks completed) for the overlapped it2-kh0
    # units: deltas as soon as their u-chunks are copied, softmax a bit
    # later so the scalar/gpsimd queues aren't head-of-line blocked.
    if K == 72:
        delta_at = {36: 0, 44: 1, 52: 2, 60: 3}
        soft_at = {48: 0, 56: 1, 64: 2}
    else:
        delta_at, soft_at = {}, {}
    done = 0
    for k0 in range(0, K, KB):
        xbd_t = kn.xbdp.tile([128, KB * 512], BF16, tag="xbd")
        nc.sync.dma_start(
            out=xbd_t[:].rearrange("p (k x) -> p k x", k=KB),
            in_=xbd_d[k0 : k0 + KB].rearrange("k p x -> p k x"),
        )
        for kk in range(KB):
            k = k0 + kk
            for pair in range(2):
                ups = kn.upsum.tile([128, 2 * CO], F32, tag="ups")
                for h in range(2):
                    bg = 2 * pair + h
                    nc.tensor.matmul(
                        ups[:, h * CO : (h + 1) * CO],
                        lhsT=xbd_t[
                            :, kk * 512 + bg * 128 : kk * 512 + (bg + 1) * 128
                        ],
                        rhs=kn.wt_t[:, k * CO : (k + 1) * CO],
                        start=True,
                        stop=True,
                    )
                dst = kn.u1v[:, 2 * pair : 2 * pair + 2, k]
                src = ups[:].rearrange("p (h x) -> p h x", h=2)
                if pair == 0:
                    nc.scalar.copy(dst, src)
                else:
                    nc.vector.tensor_copy(out=dst, in_=src)
            done += 1
            if done == (20 if K == 72 else 1):
                emit_s1()
            if done in delta_at:
                kn.unit_delta(2, delta_at[done], 0, kn.vb_a)
            if done in soft_at:
                kn.unit_soft(2, soft_at[done], 0)
    if K != 72:
        for bg in range(NBG):
            kn.unit_delta(2, bg, 0, kn.vb_a)
            kn.unit_soft(2, bg, 0)
    else:
        kn.unit_soft(2, 3, 0)

    # ---------------- iteration 2 (kh0 matmuls, then kh1) ---------------
    for bg in range(NBG):
        kn.unit_smm(2, bg, 0)
    for pr in range(2):
        kn.half_copy(2, pr)
    # Pair-pipelined iterations 2/3.  Emission order = expected ready
    # order: every pair-tail (whose DVE ops wait on a long smm->add->
    # extract DMA chain) is emitted AFTER independent delta work that
    # keeps the in-order DVE queue busy while the chain drains.
    for bg in (0, 1):
        kn.unit_delta(2, bg, 1, kn.vb_a)
        kn.unit_soft(2, bg, 1)
        kn.unit_smm(2, bg, 1)
    for bg in (2, 3):
        kn.unit_delta(2, bg, 1, kn.vb_a)
        kn.unit_soft(2, bg, 1, tail=(bg == 3))
        kn.unit_smm(2, bg, 1)
    kn.pair_tail(2, 0)
    kn.pair_v(0, kn.vb_b)
    for bg in (0, 1):
        kn.unit_delta(3, bg, 0, kn.vb_b)
        kn.unit_soft(3, bg, 0)
        kn.unit_smm(3, bg, 0)
    kn.half_copy(3, 0)
    kn.pair_tail(2, 1)
    kn.pair_v(1, kn.vb_b)
    for bg in (0, 1):
        kn.unit_delta(3, bg, 1, kn.vb_b)
        kn.unit_soft(3, bg, 1)
        kn.unit_smm(3, bg, 1)
    for bg in (2, 3):
        kn.unit_delta(3, bg, 0, kn.vb_b)
        kn.unit_soft(3, bg, 0)
        kn.unit_smm(3, bg, 0)
    kn.half_copy(3, 1)
    for bg in (2, 3):
        kn.unit_delta(3, bg, 1, kn.vb_b)
        kn.unit_soft(3, bg, 1, tail=(bg == 3))
        kn.unit_smm(3, bg, 1)
    kn.pair_tail(3, 0)
    kn.final_v(0, out_d)
    kn.pair_tail(3, 1)
    kn.final_v(1, out_d)
    if dbg is not None:
        nc.sync.dma_start(out=dbg["u1"], in_=kn.u1[:])
        nc.sync.dma_start(out=dbg["vba"], in_=kn.vb_a[:])
        nc.sync.dma_start(out=dbg["vbb"], in_=kn.vb_b[:])
        nc.sync.dma_start(out=dbg["logits"], in_=kn.logits[:])


def build(r=None):
    """Build and compile the Bass program. Returns the compiled Bacc."""
    K = _nchunks(r)
    nc = bacc.Bacc(
        "TRN2", target_bir_lowering=False, debug=False, num_devices=NCORES
    )
    xbd_d = nc.dram_tensor("xbd", [K, 128, 512], BF16, kind="ExternalInput").ap()
    wt_d = nc.dram_tensor("wt", [128, K * CO], BF16, kind="ExternalInput").ap()
    xp_d = nc.dram_tensor("xp", [128, K * BL], BF16, kind="ExternalInput").ap()
    sel_d = nc.dram_tensor("sel", [8, 128], BF16, kind="ExternalInput").ap()
    msk_d = nc.dram_tensor("msk", [128, 8], BF16, kind="ExternalInput").ap()
    out_d = nc.dram_tensor("v_out", [BL, CO], F32, kind="ExternalOutput").ap()
    dbg = None
    if _DEBUG:
        dbg = {
            "u1": nc.dram_tensor("dbg_u1", [128, NBG * K * CO], BF16, kind="ExternalOutput").ap(),
            "vba": nc.dram_tensor("dbg_vba", [128, NBG * CO], BF16, kind="ExternalOutput").ap(),
            "vbb": nc.dram_tensor("dbg_vbb", [128, NBG * CO], BF16, kind="ExternalOutput").ap(),
            "logits": nc.dram_tensor("dbg_logits", [128, NBG * K * C], F16, kind="ExternalOutput").ap(),
        }
    with nc.allow_low_precision("fp16 delta tree validated against reference"):
        with tile.TileContext(nc) as tc, ExitStack() as ctx:
            _body(ctx, tc, xbd_d, wt_d, xp_d, sel_d, msk_d, out_d, K, dbg)
    nc.compile()
    return nc


def make_inputs(x, weights, r=None):
    """Host-side marshalling: shard x over cores, rearrange to bf16 tiles."""
    K = _nchunks(r)
    r_full = K * G
    W = np.asarray(weights, dtype=np.float32)[0][:r_full]  # [R, C, IC, OC]
    wt = (
        W.reshape(K, G, C, IC, OC)
        .transpose(0, 1, 3, 2, 4)
        .reshape(K, 128, CO)
        .transpose(1, 0, 2)
        .reshape(128, K * CO)
        .astype(NPBF)
    )
    sel = np.zeros((8, 128), dtype=np.float32)
    bi = np.arange(8)
    gi = np.arange(G)
    sel[bi[:, None], bi[:, None] * G + gi[None, :]] = 1.0
    sel = sel.astype(NPBF)
    msk = np.zeros((128, 8), dtype=np.float32)
    pi = np.arange(128)
    msk[pi, pi // G] = 1.0
    msk = msk.astype(NPBF)

    in_maps = []
    xf = np.asarray(x, dtype=np.float32)[:, :r_full]
    for core in range(NCORES):
        xl = xf[core * BL : (core + 1) * BL]  # [BL, R, IC]
        xr = xl.transpose(1, 2, 0).reshape(K, G, IC, BL)  # [K, g, i, b]
        xp = (
            xr.reshape(K, 128, BL).transpose(1, 0, 2).reshape(128, K * BL)
        ).astype(NPBF)
        xrg = xr.reshape(K, G, IC, NBG, 8)
        xbd6 = np.zeros((K, G, IC, NBG, 8, G), dtype=np.float32)
        for g in range(G):
            xbd6[:, g, :, :, :, g] = xrg[:, g]
        xbd = xbd6.reshape(K, 128, 512).astype(NPBF)
        in_maps.append(
            {"xbd": xbd, "wt": wt, "xp": xp, "sel": sel, "msk": msk}
        )
    return in_maps


_CACHE = {}


def kernel(x, weights):
    if "nc" not in _CACHE:
        _CACHE["nc"] = build()
    nc = _CACHE["nc"]
    in_maps = make_inputs(x, weights)
    res = run_bass_kernel_spmd(nc, in_maps, core_ids=list(range(NCORES)))
    outs = [res.results[i]["v_out"].reshape(BL, C, OC) for i in range(NCORES)]
    return np.concatenate(outs, axis=0)


# revision 24
# speedup vs baseline: 1.0586x; 1.0586x over previous
"""DigitCaps dynamic-routing kernel for Trainium2 (8 NeuronCores, SPMD).

Problem:  u = einsum('bri,rcio->brco', x, W[0]);  3 routing iterations
          (softmax over capsules, weighted sum over routes, squash,
          agreement update);  returns v [B, C, OC].

Shapes: B=256, R=1152, C=10, IC=8, OC=16.  Batch-sharded 8 ways (BL=32
per core, zero cross-core communication).

Design notes (per core):
 - u-phase: r in 72 chunks of G=16 routes; block-diag x (xbd) gives
   128-partition matmuls so each chunk's u lands as 4x [128,160] PSUM
   tiles; u resident in SBUF as bf16 [p=(b8,g16), f=(bg4,k72,c10,o16)]
   (bg-major so the delta-pass reads are contiguous).
 - iter-1 shortcut: c uniform = 1/10, so s1 = 0.1*sum_r u comes from 72
   accumulating matmuls with compact x (xp) as lhsT.  wt/xp load in one
   big DMA each up-front, so v1 is ready ~20us in and iteration 2
   overlaps the u-phase.
 - delta (b-update, sum_o u*v): DVE fp16 multiply at 2x mode + fp16
   add-tree (16->8->4->2->1), also at 2x.  The old tensor_reduce ran at
   1x (InstTensorReduce supports no DVE perf modes); the tree is ~1.7x
   faster end to end.  Logits accumulate in fp16 (|b| <~ 60, quantum
   ~0.03 -- checked against the 2e-2 gate).
 - u PSUM->SBUF copies run on scalar+gpsimd so the DVE is free for the
   overlapped iteration-2 work.
 - squash: sqrt as exp(0.5*ln(n2+eps^2)); ln/exp/copy share one
   activation table (natural_log_exp_and_others) so the scalar engine
   never reloads tables mid-kernel (the old sqrt<->exp alternation cost
   ~1.3us per switch).
 - s-pass: block-diag c (cbd = c x batch-mask, with 1/softmax-sum folded
   into the mask) as lhsT against resident u, PSUM-accumulated over all
   72 chunks; (c,c') diagonal extracted with small DMAs.  cbd builds go
   to gpsimd except the final tail unit (vector, to shorten the tail).
 - PSUM budget (8 banks): 4x u-tiles + s1 + 2 paired s-tiles + vb.
"""

import sys

sys.path.insert(0, "/opt/trn_rl_repo")

from contextlib import ExitStack

import ml_dtypes
import numpy as np

import concourse.bass as bass
import concourse.tile as tile
from concourse import bacc, mybir
from concourse.bass_utils import run_bass_kernel_spmd

BF16 = mybir.dt.bfloat16
F16 = mybir.dt.float16
F32 = mybir.dt.float32
AF = mybir.ActivationFunctionType
ALU = mybir.AluOpType
AX = mybir.AxisListType

B, R, C, IC, OC = 256, 1152, 10, 8, 16
NCORES = 8
BL = B // NCORES  # 32 batches per core
G = 16  # routes per chunk
NBG = BL // 8  # 4 b-groups of 8
CO = C * OC  # 160
EPS2 = 1e-16
NPBF = ml_dtypes.bfloat16

# Set by tests to shrink the problem for simulation; full size by default.
_R_OVERRIDE = None
_DEBUG = False


def _nchunks(r=None):
    r = r if r is not None else (_R_OVERRIDE or R)
    assert r % G == 0
    return r // G


class _Kern:
    def __init__(self, ctx, tc, K):
        self.nc = tc.nc
        self.K = K
        self.KH = K // 2
        self.per = ctx.enter_context(tc.tile_pool(name="per", bufs=1))
        self.xbdp = ctx.enter_context(tc.tile_pool(name="xbdp", bufs=2))
        self.upsum = ctx.enter_context(
            tc.tile_pool(name="upsum", bufs=4, space="PSUM")
        )
        self.s1psum = ctx.enter_context(
            tc.tile_pool(name="s1psum", bufs=1, space="PSUM")
        )
        self.spsum = ctx.enter_context(
            tc.tile_pool(name="spsum", bufs=1, space="PSUM")
        )
        self.vbpsum = ctx.enter_context(
            tc.tile_pool(name="vbpsum", bufs=1, space="PSUM")
        )
        self.tmpp = ctx.enter_context(tc.tile_pool(name="tmpp", bufs=1))
        self.trp = ctx.enter_context(tc.tile_pool(name="trp", bufs=1))
        self.cbdp = ctx.enter_context(tc.tile_pool(name="cbdp", bufs=3))
        self.cxp = ctx.enter_context(tc.tile_pool(name="cxp", bufs=3))
        self.small = ctx.enter_context(tc.tile_pool(name="small", bufs=2))

        K_, KH = K, self.KH
        per = self.per
        self.u1 = per.tile([128, NBG * K_ * CO], BF16, name="u1")
        self.u1v = self.u1[:].rearrange("p (b k x) -> p b k x", b=NBG, k=K_)
        self.logits = per.tile([128, NBG * K_ * C], F16, name="logits")
        self.logv = self.logits[:].rearrange(
            "p (b k c) -> p b k c", b=NBG, k=K_
        )
        self.wt_t = per.tile([128, K_ * CO], BF16, name="wt_t")
        self.xp_t = per.tile([128, K_ * BL], BF16, name="xp_t")
        self.sel_t = per.tile([8, 128], BF16, name="sel_t")
        self.msk_t = per.tile([128, 8], BF16, name="msk_t")
        self.s_sb = per.tile([BL, CO], F32, name="s_sb")
        self.s8 = per.tile([8, NBG * CO], F32, name="s8")
        self.vbf8 = per.tile([8, NBG * CO], BF16, name="vbf8")
        self.vb_a = per.tile([128, NBG * CO], BF16, tag="vb_a", name="vb_a")
        self.vb_b = per.tile([128, NBG * CO], BF16, tag="vb_b", name="vb_b")
        self.s84a = per.tile([8, NBG * CO], F32, name="s84a")
        self.s84b = per.tile([8, NBG * CO], F32, name="s84b")
        self.v84 = per.tile([8, NBG * CO], F32, name="v84")
        self.vbf8b = per.tile([8, NBG * CO], BF16, name="vbf8b")
        self.epsb = per.tile([128, 1], F32, name="epsb")
        self.nc.vector.memset(self.epsb[:], EPS2)
        self.sps = {}  # (it, pair) -> [80, 2*CO] PSUM tile
        self.cbds = {}  # (it, bg, kh) -> cbd view
        self.sbg = {}  # (it, bg) -> extracted s tile

    # -- squash ----------------------------------------------------------
    def squash(self, s_ap, v_ap, pre, np_, nseg):
        """v = squash(pre*s) for [np_, nseg*OC] tiles at partition base 0."""
        nc, pool = self.nc, self.small
        sq = pool.tile([np_, nseg * OC], F32, tag="sq", bufs=1)
        if pre == 1.0:
            nc.vector.tensor_tensor(out=sq[:], in0=s_ap, in1=s_ap, op=ALU.mult)
        else:
            nc.vector.scalar_tensor_tensor(
                out=sq[:], in0=s_ap, scalar=pre * pre, in1=s_ap,
                op0=ALU.mult, op1=ALU.mult,
            )
        n2 = pool.tile([np_, nseg], F32, tag="n2", bufs=1)
        nc.vector.reduce_sum(
            out=n2[:], in_=sq[:].rearrange("p (c o) -> p c o", c=nseg), axis=AX.X
        )
        # nrm = sqrt(n2 + EPS2) via exp(0.5*ln(.)): stays in one act table
        nrm = pool.tile([np_, nseg], F32, tag="nrm", bufs=1)
        nc.scalar.activation(nrm[:], n2[:], AF.Ln, bias=self.epsb[:np_])
        nc.scalar.activation(nrm[:], nrm[:], AF.Exp, scale=0.5)
        t1 = pool.tile([np_, nseg], F32, tag="t1", bufs=1)
        nc.vector.tensor_scalar(
            out=t1[:], in0=n2[:], scalar1=1.0, scalar2=None, op0=ALU.add
        )
        den = pool.tile([np_, nseg], F32, tag="den", bufs=1)
        nc.vector.tensor_tensor(out=den[:], in0=nrm[:], in1=t1[:], op=ALU.mult)
        rden = pool.tile([np_, nseg], F32, tag="rden", bufs=1)
        nc.vector.reciprocal_approx_fast(rden[:], den[:])
        sc = pool.tile([np_, nseg], F32, tag="sc", bufs=1)
        if pre == 1.0:
            nc.vector.tensor_tensor(
                out=sc[:], in0=n2[:], in1=rden[:], op=ALU.mult
            )
        else:
            nc.vector.scalar_tensor_tensor(
                out=sc[:], in0=n2[:], scalar=pre, in1=rden[:],
                op0=ALU.mult, op1=ALU.mult,
            )
        nc.vector.tensor_tensor(
            out=v_ap.rearrange("p (c o) -> p c o", c=nseg),
            in0=s_ap.rearrange("p (c o) -> p c o", c=nseg),
            in1=sc[:].unsqueeze(2).broadcast_to([np_, nseg, OC]),
            op=ALU.mult,
        )

    # -- one routing unit: delta ----------------------------------------
    def unit_delta(self, it, bg, kh, vb):
        """fp16 delta = sum_o u*v via 2x multiply + 2x add-tree."""
        nc, KH = self.nc, self.KH
        ks = kh * KH
        tmpt = self.tmpp.tile([128, KH * CO], F16, tag="tmp")
        nc.vector.tensor_tensor(
            out=tmpt[:].rearrange("p (k x) -> p k x", k=KH),
            in0=self.u1v[:, bg, ks : ks + KH],
            in1=vb[:, bg * CO : (bg + 1) * CO]
            .unsqueeze(1)
            .broadcast_to([128, KH, CO]),
            op=ALU.mult,
        )
        tv = tmpt[:].rearrange("p (s o) -> p s o", o=16)
        t8 = self.trp.tile([128, KH * C * 8], F16, tag="t8")
        t8v = t8[:].rearrange("p (s o) -> p s o", o=8)
        nc.vector.tensor_tensor(
            out=t8v, in0=tv[:, :, 0:8], in1=tv[:, :, 8:16], op=ALU.add
        )
        t4 = self.trp.tile([128, KH * C * 4], F16, tag="t4")
        t4v = t4[:].rearrange("p (s o) -> p s o", o=4)
        nc.vector.tensor_tensor(
            out=t4v, in0=t8v[:, :, 0:4], in1=t8v[:, :, 4:8], op=ALU.add
        )
        t2 = self.trp.tile([128, KH * C * 2], F16, tag="t2")
        t2v = t2[:].rearrange("p (s o) -> p s o", o=2)
        nc.vector.tensor_tensor(
            out=t2v, in0=t4v[:, :, 0:2], in1=t4v[:, :, 2:4], op=ALU.add
        )
        lh = self.logv[:, bg, ks : ks + KH].rearrange("p k c -> p (k c)")
        if it == 2:
            nc.vector.tensor_tensor(
                out=lh, in0=t2v[:, :, 0], in1=t2v[:, :, 1], op=ALU.add
            )
        else:
            dtm = self.trp.tile([128, KH * C], F16, tag="dtm")
            nc.vector.tensor_tensor(
                out=dtm[:], in0=t2v[:, :, 0], in1=t2v[:, :, 1], op=ALU.add
            )
            nc.vector.tensor_tensor(out=lh, in0=lh, in1=dtm[:], op=ALU.add)

    # -- one routing unit: softmax + block-diag c ------------------------
    def unit_soft(self, it, bg, kh, tail=False):
        nc, KH = self.nc, self.KH
        ks = kh * KH
        lh = self.logv[:, bg, ks : ks + KH]
        ch = self.cxp.tile([128, KH * C], BF16, tag="ch")
        chv = ch[:].rearrange("p (k c) -> p k c", k=KH)
        nc.scalar.activation(chv, lh, AF.Exp)
        sume = self.small.tile([128, KH], F32, tag="sume")
        nc.vector.reduce_sum(out=sume[:], in_=chv, axis=AX.X)
        rs = self.small.tile([128, KH], F32, tag="rs")
        nc.vector.reciprocal_approx_fast(rs[:], sume[:])
        rsb = self.small.tile([128, KH], BF16, tag="rsb")
        nc.scalar.copy(rsb[:], rs[:])
        # fold 1/sum into the batch mask, then build block-diag c
        eng = nc.vector if tail else nc.gpsimd
        msk2 = self.small.tile([128, KH * 8], BF16, tag="msk2")
        m2v = msk2[:].rearrange("p (k e) -> p k e", k=KH)
        eng.tensor_tensor(
            out=m2v,
            in0=self.msk_t[:].unsqueeze(1).broadcast_to([128, KH, 8]),
            in1=rsb[:].unsqueeze(2).broadcast_to([128, KH, 8]),
            op=ALU.mult,
        )
        cbd_t = self.cbdp.tile([128, KH * C * 8], BF16, tag="cbd")
        cbdv = cbd_t[:].rearrange("p (k c e) -> p k c e", k=KH, c=C)
        eng.tensor_tensor(
            out=cbdv,
            in0=chv.unsqueeze(3).broadcast_to([128, KH, C, 8]),
            in1=m2v.unsqueeze(2).broadcast_to([128, KH, C, 8]),
            op=ALU.mult,
        )
        self.cbds[(it, bg, kh)] = cbdv

    # -- one routing unit: s-pass matmuls --------------------------------
    # PSUM rule (found the hard way): only ONE open accumulation group per
    # bank -- a start=True while another group in the same bank is still
    # open wipes that group's partials.  Each (bg, kh) chain is therefore
    # its own closed start..stop group; the kh0 result is copied to SBUF
    # before the kh1 groups reuse the bank, and the halves are added there.
    def unit_smm(self, it, bg, kh):
        nc, KH = self.nc, self.KH
        ks = kh * KH
        pair = bg // 2
        if (it, pair) not in self.sps:
            self.sps[(it, pair)] = self.spsum.tile(
                [80, 2 * CO], F32, tag=f"sps{pair}", name=f"sps{it}{pair}"
            )
        half = bg % 2
        sps = self.sps[(it, pair)][:, half * CO : (half + 1) * CO]
        cbdv = self.cbds[(it, bg, kh)]
        for kk in range(KH):
            nc.tensor.matmul(
                sps,
                lhsT=cbdv[:, kk].rearrange("p c e -> p (c e)"),
                rhs=self.u1v[:, bg, ks + kk],
                start=(kk == 0),
                stop=(kk == KH - 1),
            )

    def half_copy(self, it, pr):
        """copy a bg-pair's kh0 s-partials out of PSUM so the bank can host
        the kh1 accumulation groups."""
        h0 = self.small.tile([80, 2 * CO], F32, tag=f"half{pr}", bufs=1)
        self.nc.scalar.copy(h0[:], self.sps[(it, pr)][:])
        self.halfs = getattr(self, "halfs", {})
        self.halfs[(it, pr)] = h0

    def pair_tail(self, it, pr):
        """kh1 + kh0 combine (DVE add straight from PSUM) and (c,c')
        diagonal extraction for a bg-pair."""
        nc = self.nc
        sfull = self.small.tile([80, 2 * CO], F32, tag="sfull")
        nc.vector.tensor_tensor(
            out=sfull[:],
            in0=self.sps[(it, pr)][:],
            in1=self.halfs[(it, pr)][:],
            op=ALU.add,
        )
        s84 = self.s84a if it == 2 else self.s84b
        s84v = s84[:].rearrange("p (b c o) -> p b c o", b=NBG, c=C)
        sfv = sfull[:].rearrange("p (h c o) -> p h c o", h=2, c=C)
        qs = (nc.sync, nc.gpsimd, nc.scalar)
        for c in range(C):
            qs[c % 3].dma_start(
                out=s84v[:, 2 * pr : 2 * pr + 2, c],
                in_=sfv[c * 8 : (c + 1) * 8, :, c],
            )

    def pair_v(self, pr, vb_dst):
        """squash one bg-pair's extracted s -> broadcast into vb (two bgs
        batched: two Ln/Exp pairs per iteration instead of four)."""
        nc = self.nc
        sl = slice(2 * pr * CO, (2 * pr + 2) * CO)
        self.squash(
            self.s84a[:, sl], self.vbf8b[:, sl], pre=1.0, np_=8, nseg=2 * C
        )
        for bg in (2 * pr, 2 * pr + 1):
            vbp = self.vbpsum.tile([128, CO], F32, tag="vbp")
            nc.tensor.matmul(
                vbp[:],
                lhsT=self.sel_t[:],
                rhs=self.vbf8b[:, bg * CO : (bg + 1) * CO],
                start=True,
                stop=True,
            )
            nc.scalar.copy(vb_dst[:, bg * CO : (bg + 1) * CO], vbp[:])

    def final_v(self, pr, out_d):
        """squash one bg-pair of iteration 3 straight to the DRAM output."""
        nc = self.nc
        sl = slice(2 * pr * CO, (2 * pr + 2) * CO)
        self.squash(
            self.s84b[:, sl], self.v84[:, sl], pre=1.0, np_=8, nseg=2 * C
        )
        for bg in (2 * pr, 2 * pr + 1):
            nc.sync.dma_start(
                out=out_d[bg * 8 : (bg + 1) * 8, :],
                in_=self.v84[:, bg * CO : (bg + 1) * CO],
            )


def _body(ctx, tc, xbd_d, wt_d, xp_d, sel_d, msk_d, out_d, K, dbg=None):
    nc = tc.nc
    kn = _Kern(ctx, tc, K)
    KH = kn.KH

    # wt (halves, for an early s1 start) on the scalar queue; xbd streams
    # on sync; all small early DMAs on gpsimd so nothing queues behind the
    # big streams (per-queue FIFO).
    half = K // 2 * CO
    # warm both activation tables (ln-set / exp-set) before the hot path
    nc.scalar.activation(kn.epsb[:8], kn.epsb[:8], AF.Ln)
    nc.scalar.activation(kn.epsb[:8], kn.epsb[:8], AF.Exp)
    nc.vector.memset(kn.epsb[:], EPS2)
    nc.scalar.dma_start(out=kn.wt_t[:, :half], in_=wt_d[:, :half])
    nc.scalar.dma_start(out=kn.wt_t[:, half:], in_=wt_d[:, half:])
    nc.gpsimd.dma_start(out=kn.xp_t[:], in_=xp_d)
    nc.gpsimd.dma_start(out=kn.sel_t[:], in_=sel_d)
    nc.gpsimd.dma_start(out=kn.msk_t[:], in_=msk_d)

    # ---------------- iteration-1 s (emitted mid-u-phase) ---------------
    def emit_s1():
        s1ps = kn.s1psum.tile([BL, CO], F32, tag="s1")
        for k in range(K):
            nc.tensor.matmul(
                s1ps[:],
                lhsT=kn.xp_t[:, k * BL : (k + 1) * BL],
                rhs=kn.wt_t[:, k * CO : (k + 1) * CO],
                start=(k == 0),
                stop=(k == K - 1),
            )
        nc.scalar.copy(kn.s_sb[:], s1ps[:])
        for bg in range(NBG):
            nc.gpsimd.dma_start(
                out=kn.s8[:, bg * CO : (bg + 1) * CO],
                in_=kn.s_sb[bg * 8 : (bg + 1) * 8, :],
            )
        kn.squash(kn.s8[:], kn.vbf8[:], pre=1.0 / C, np_=8, nseg=NBG * C)
        for bg in range(NBG):
            vbp = kn.vbpsum.tile([128, CO], F32, tag="vbp")
            nc.tensor.matmul(
                vbp[:],
                lhsT=kn.sel_t[:],
                rhs=kn.vbf8[:, bg * CO : (bg + 1) * CO],
                start=True,
                stop=True,
            )
            nc.scalar.copy(kn.vb_a[:, bg * CO : (bg + 1) * CO], vbp[:])

    # ---------------- u-phase (iteration-2 kh0 work interleaved) --------
    assert K % 4 == 0 or K < 4
    KB = 4 if K % 4 == 0 else 1
    # emission points (in chunks completed) for the overlapped it2-kh0
    # units: deltas as soon as their u-chunks are copied, softmax a bit
    # later so the scalar/gpsimd queues aren't head-of-line blocked.
    if K == 72:
        delta_at = {36: 0, 44: 1, 52: 2, 60: 3}
        soft_at = {48: 0, 56: 1, 64: 2}
    else:
        delta_at, soft_at = {}, {}
    done = 0
    for k0 in range(0, K, KB):
        xbd_t = kn.xbdp.tile([128, KB * 512], BF16, tag="xbd")
        nc.sync.dma_start(
            out=xbd_t[:].rearrange("p (k x) -> p k x", k=KB),
            in_=xbd_d[k0 : k0 + KB].rearrange("k p x -> p k x"),
        )
        for kk in range(KB):
            k = k0 + kk
            for pair in range(2):
                ups = kn.upsum.tile([128, 2 * CO], F32, tag="ups")
                for h in range(2):
                    bg = 2 * pair + h
                    nc.tensor.matmul(
                        ups[:, h * CO : (h + 1) * CO],
                        lhsT=xbd_t[
                            :, kk * 512 + bg * 128 : kk * 512 + (bg + 1) * 128
                        ],
                        rhs=kn.wt_t[:, k * CO : (k + 1) * CO],
                        start=True,
                        stop=True,
                    )
                dst = kn.u1v[:, 2 * pair : 2 * pair + 2, k]
                src = ups[:].rearrange("p (h x) -> p h x", h=2)
                if pair == 0 or k >= 36:
                    nc.scalar.copy(dst, src)
                else:
                    nc.vector.tensor_copy(out=dst, in_=src)
            done += 1
            if done == (20 if K == 72 else 1):
                emit_s1()
            if done in delta_at:
                kn.unit_delta(2, delta_at[done], 0, kn.vb_a)
            if done in soft_at:
                kn.unit_soft(2, soft_at[done], 0)
    if K != 72:
        for bg in range(NBG):
            kn.unit_delta(2, bg, 0, kn.vb_a)
            kn.unit_soft(2, bg, 0)
    else:
        kn.unit_soft(2, 3, 0)

    # ---------------- iteration 2 (kh0 matmuls, then kh1) ---------------
    for bg in range(NBG):
        kn.unit_smm(2, bg, 0)
    for pr in range(2):
        kn.half_copy(2, pr)
    # Pair-pipelined iterations 2/3.  Emission order = expected ready
    # order: every pair-tail (whose DVE ops wait on a long smm->add->
    # extract DMA chain) is emitted AFTER independent delta work that
    # keeps the in-order DVE queue busy while the chain drains.
    for bg in (0, 1):
        kn.unit_delta(2, bg, 1, kn.vb_a)
        kn.unit_soft(2, bg, 1)
        kn.unit_smm(2, bg, 1)
    for bg in (2, 3):
        kn.unit_delta(2, bg, 1, kn.vb_a)
        kn.unit_soft(2, bg, 1, tail=(bg == 3))
        kn.unit_smm(2, bg, 1)
    kn.pair_tail(2, 0)
    kn.pair_v(0, kn.vb_b)
    for bg in (0, 1):
        kn.unit_delta(3, bg, 0, kn.vb_b)
        kn.unit_soft(3, bg, 0)
        kn.unit_smm(3, bg, 0)
    kn.half_copy(3, 0)
    kn.pair_tail(2, 1)
    kn.pair_v(1, kn.vb_b)
    for bg in (0, 1):
        kn.unit_delta(3, bg, 1, kn.vb_b)
        kn.unit_soft(3, bg, 1)
        kn.unit_smm(3, bg, 1)
    for bg in (2, 3):
        kn.unit_delta(3, bg, 0, kn.vb_b)
        kn.unit_soft(3, bg, 0)
        kn.unit_smm(3, bg, 0)
    kn.half_copy(3, 1)
    for bg in (2, 3):
        kn.unit_delta(3, bg, 1, kn.vb_b)
        kn.unit_soft(3, bg, 1, tail=(bg == 3))
        kn.unit_smm(3, bg, 1)
    kn.pair_tail(3, 0)
    kn.final_v(0, out_d)
    kn.pair_tail(3, 1)
    kn.final_v(1, out_d)
    if dbg is not None:
        nc.sync.dma_start(out=dbg["u1"], in_=kn.u1[:])
        nc.sync.dma_start(out=dbg["vba"], in_=kn.vb_a[:])
        nc.sync.dma_start(out=dbg["vbb"], in_=kn.vb_b[:])
        nc.sync.dma_start(out=dbg["logits"], in_=kn.logits[:])


def build(r=None):
    """Build and compile the Bass program. Returns the compiled Bacc."""
    K = _nchunks(r)
    nc = bacc.Bacc(
        "TRN2", target_bir_lowering=False, debug=False, num_devices=NCORES
    )
    xbd_d = nc.dram_tensor("xbd", [K, 128, 512], BF16, kind="ExternalInput").ap()
    wt_d = nc.dram_tensor("wt", [128, K * CO], BF16, kind="ExternalInput").ap()
    xp_d = nc.dram_tensor("xp", [128, K * BL], BF16, kind="ExternalInput").ap()
    sel_d = nc.dram_tensor("sel", [8, 128], BF16, kind="ExternalInput").ap()
    msk_d = nc.dram_tensor("msk", [128, 8], BF16, kind="ExternalInput").ap()
    out_d = nc.dram_tensor("v_out", [BL, CO], F32, kind="ExternalOutput").ap()
    dbg = None
    if _DEBUG:
        dbg = {
            "u1": nc.dram_tensor("dbg_u1", [128, NBG * K * CO], BF16, kind="ExternalOutput").ap(),
            "vba": nc.dram_tensor("dbg_vba", [128, NBG * CO], BF16, kind="ExternalOutput").ap(),
            "vbb": nc.dram_tensor("dbg_vbb", [128, NBG * CO], BF16, kind="ExternalOutput").ap(),
            "logits": nc.dram_tensor("dbg_logits", [128, NBG * K * C], F16, kind="ExternalOutput").ap(),
        }
    with nc.allow_low_precision("fp16 delta tree validated against reference"):
        with tile.TileContext(nc) as tc, ExitStack() as ctx:
            _body(ctx, tc, xbd_d, wt_d, xp_d, sel_d, msk_d, out_d, K, dbg)
    nc.compile()
    return nc


def make_inputs(x, weights, r=None):
    """Host-side marshalling: shard x over cores, rearrange to bf16 tiles."""
    K = _nchunks(r)
    r_full = K * G
    W = np.asarray(weights, dtype=np.float32)[0][:r_full]  # [R, C, IC, OC]
    wt = (
        W.reshape(K, G, C, IC, OC)
        .transpose(0, 1, 3, 2, 4)
        .reshape(K, 128, CO)
        .transpose(1, 0, 2)
        .reshape(128, K * CO)
        .astype(NPBF)
    )
    sel = np.zeros((8, 128), dtype=np.float32)
    bi = np.arange(8)
    gi = np.arange(G)
    sel[bi[:, None], bi[:, None] * G + gi[None, :]] = 1.0
    sel = sel.astype(NPBF)
    msk = np.zeros((128, 8), dtype=np.float32)
    pi = np.arange(128)
    msk[pi, pi // G] = 1.0
    msk = msk.astype(NPBF)

    in_maps = []
    xf = np.asarray(x, dtype=np.float32)[:, :r_full]
    for core in range(NCORES):
        xl = xf[core * BL : (core + 1) * BL]  # [BL, R, IC]
        xr = xl.transpose(1, 2, 0).reshape(K, G, IC, BL)  # [K, g, i, b]
        xp = (
            xr.reshape(K, 128, BL).transpose(1, 0, 2).reshape(128, K * BL)
        ).astype(NPBF)
        xrg = xr.reshape(K, G, IC, NBG, 8)
        xbd6 = np.zeros((K, G, IC, NBG, 8, G), dtype=np.float32)
        for g in range(G):
            xbd6[:, g, :, :, :, g] = xrg[:, g]
        xbd = xbd6.reshape(K, 128, 512).astype(NPBF)
        in_maps.append(
            {"xbd": xbd, "wt": wt, "xp": xp, "sel": sel, "msk": msk}
        )
    return in_maps


_CACHE = {}


def kernel(x, weights):
    if "nc" not in _CACHE:
        _CACHE["nc"] = build()
    nc = _CACHE["nc"]
    in_maps = make_inputs(x, weights)
    res = run_bass_kernel_spmd(nc, in_maps, core_ids=list(range(NCORES)))
    outs = [res.results[i]["v_out"].reshape(BL, C, OC) for i in range(NCORES)]
    return np.concatenate(outs, axis=0)
